# revision 11
# baseline (speedup 1.0000x reference)
"""Trainium2 Bass kernel for nn_MinimumErrorRateLoss.

Computes, for logits (B,P,H,C), ref (B,P,R), hyp (B,P,H):
    loss = mean_{b,p}[ (er - mean_p er) * softmax_p(log_probs) + 0.01 * ce ]
where
    er        = levenshtein(ref, hyp) / R
    log_probs = sum_h (logits[h, hyp[h]] - logsumexp_c logits[h, :])
    ce        = sum_{s<100} (logsumexp_c logits[s, :] - logits[s, ref[s]])

Sharding: data-parallel over the batch dim across 8 NeuronCores (4 batches
each).  Per core the kernel:
  * streams its 64 (b,p) tiles of [128,1024] logits through SBUF in
    4-tile/2MB DMAs; ScalarE computes exp (no max-subtraction needed for
    ~N(0,1) logits) with a fused free-dim accumulate into a PSUM scratch
    (-> logsumexp after one Ln),
  * extracts the hyp/ref-indexed logits elements with one GPSIMD ap_gather
    per 4-tile batch (per 16-partition group: 16 hyp + 16 ref indices per
    tile, host-pre-biased into the 4-tile window), followed by one
    full-width masked multiply + reduce on VectorE and ones/first-100
    vector matmuls on TensorE (per-element indirect DMA is not viable:
    HW consumes one offset per partition per instruction),
  * runs the edit-distance DP on VectorE in fp16 (all values are small
    integers, exact) with two chained instructions per hyp step (a fused
    scalar_tensor_tensor add and a tensor_tensor_scan max-recurrence),
    using the transformation Y[i,j] = j + i - D[i,j] which turns the row
    update into
        Y_i[j] = max(Y_{i-1}[j-1] + 1 + eq[i,j], Y_i[j-1], Y_{i-1}[j])
    with the boundary column Y_i[0] = 0 for all i, so the scan's initial
    value is the compile-time constant 0.

Measured (8 axon vNCs): relative error 5.1e-07 vs the jax reference;
steady-state ~42-50 us/iteration (reps-slope), vs a ~46 us HBM floor at
the ~735 GB/s effective per-core bandwidth observed on these vNCs.
"""

import numpy as np

B, P, H, R, C = 32, 16, 128, 100, 1024
NCORES = 8
BL = B // NCORES  # local batches per core
NT = BL * P       # tiles (sequences) per core

_CACHE = {}


def _build_program(reps=1, _skip=()):
    import concourse.bass as bass
    import concourse.bacc as bacc
    import concourse.tile as tile
    import concourse.mybir as mybir

    f32 = mybir.dt.float32
    Alu = mybir.AluOpType
    Act = mybir.ActivationFunctionType

    nc = bacc.Bacc("TRN2", target_bir_lowering=False, debug=False)

    logits_d = nc.dram_tensor("logits", [NT, H, C], f32, kind="ExternalInput")
    ref_d = nc.dram_tensor("ref_f32", [NT, R], f32, kind="ExternalInput")
    hyp_d = nc.dram_tensor("hyp_f32", [NT, H], f32, kind="ExternalInput")
    idx_d = nc.dram_tensor("idx16", [H, 2 * NT], mybir.dt.int16,
                           kind="ExternalInput")
    mask_d = nc.dram_tensor("mask", [H, 2], f32, kind="ExternalInput")
    gmask_d = nc.dram_tensor("gmask", [H, 32], f32, kind="ExternalInput")
    out_d = nc.dram_tensor("contrib", [BL, P], f32, kind="ExternalOutput")

    with tile.TileContext(nc) as tc:
        with (
            tc.tile_pool(name="persist", bufs=1) as pp,
            tc.tile_pool(name="lt", bufs=4) as ltp,
            tc.tile_pool(name="scratch", bufs=2, space="PSUM") as scp,
            tc.tile_pool(name="psum", bufs=1, space="PSUM") as psp,
        ):
            for _rep in range(reps):
                _emit_body(nc, bass, mybir, f32, Alu, Act,
                           logits_d, ref_d, hyp_d, idx_d, mask_d, gmask_d,
                           out_d, pp, ltp, scp, psp, _skip)

    nc.compile()
    return nc


def _emit_body(nc, bass, mybir, f32, Alu, Act,
               logits_d, ref_d, hyp_d, idx_d, mask_d, gmask_d, out_d,
               pp, ltp, scp, psp, _skip=()):
    AxX = mybir.AxisListType.X

    # ---------------- DP inputs and serial chain (VectorE) ----------
    ref_sb = pp.tile([NT, R], f32)
    hyp_sb = pp.tile([NT, H], f32)
    nc.sync.dma_start(out=ref_sb[:], in_=ref_d[:])
    nc.sync.dma_start(out=hyp_sb[:], in_=hyp_d[:])

    f16 = mybir.dt.float16
    eqm = pp.tile([NT, H, R], f16)
    ra, ha = ref_sb[:], hyp_sb[:]
    # eqm[t, s, j] = (ref[t, j] == hyp[t, s])
    ref_bc = bass.AP(tensor=ra.tensor, offset=ra.offset,
                     ap=[ra.ap[0], [0, H], ra.ap[1]])
    hyp_bc = bass.AP(tensor=ha.tensor, offset=ha.offset,
                     ap=[ha.ap[0], ha.ap[1], [0, R]])
    if "eq" not in _skip:
        nc.vector.tensor_tensor(out=eqm[:], in0=ref_bc, in1=hyp_bc,
                                op=Alu.is_equal)
    else:
        nc.vector.memset(eqm[:], 0.0)

    ya = pp.tile([NT, R + 1], f16)
    yb = pp.tile([NT, R + 1], f16)
    ab = pp.tile([NT, R], f16)
    nc.vector.memset(ya[:], 0.0)
    nc.vector.memset(yb[:, 0:1], 0.0)

    bufs = [ya, yb]
    for s in range(H if "dp" not in _skip else 0):
        yp = bufs[s % 2]
        yn = bufs[(s + 1) % 2]
        # A[j] = Yprev[j-1] + 1 + eq[s, j],   j = 1..R
        nc.vector.scalar_tensor_tensor(
            out=ab[:], in0=yp[:, 0:R], scalar=1.0, in1=eqm[:, s, :],
            op0=Alu.add, op1=Alu.add)
        # Ynew[j] = max(A[j], Ynew[j-1], Yprev[j]),  Ynew[0] = 0
        nc.vector.tensor_tensor_scan(
            out=yn[:, 1:R + 1], data0=ab[:], data1=yp[:, 1:R + 1],
            initial=0.0, op0=Alu.max, op1=Alu.max)

    yfin = bufs[H % 2]
    pack = pp.tile([NT, 4], f32)
    # er = dist/R = (R + H - Y)/R
    nc.vector.tensor_scalar(
        out=pack[:, 0:1], in0=yfin[:, R:R + 1],
        scalar1=-1.0 / R, scalar2=float(R + H) / R,
        op0=Alu.mult, op1=Alu.add)

    # ------------- logsumexp stream + per-tile gathers ---------------
    idx_sb = pp.tile([H, 2 * NT], mybir.dt.int16)
    nc.sync.dma_start(out=idx_sb[:], in_=idx_d[:])
    gbuf = pp.tile([H, NT, 32], f32)
    sumexp = pp.tile([H, NT], f32)
    if "gather" in _skip:
        nc.vector.memset(gbuf[:], 0.0)
    TB = 4  # tiles per DMA / ap_gather batch
    lgap = logits_d.ap()
    for t0 in range(0, NT, TB):
        lt = ltp.tile([H, TB, C], f32)
        # DRAM [t, h, c] -> SBUF [h, t, c]
        src_ap = bass.AP(tensor=lgap.tensor, offset=t0 * H * C,
                         ap=[[C, H], [H * C, TB], [1, C]])
        nc.sync.dma_start(out=lt[:], in_=src_ap)
        for tt in range(TB):
            t = t0 + tt
            if "act" not in _skip:
                sc = scp.tile([H, C], f32, space="PSUM")
                nc.scalar.activation(out=sc[:], in_=lt[:, tt, :],
                                     func=Act.Exp,
                                     accum_out=sumexp[:, t:t + 1])
            elif t == 0:
                nc.vector.memset(sumexp[:], 1.0)
        # out[h, tt, i<16] = lt[h, tt, hyp[t0+tt, 16*(h//16)+i]]  (idxs
        # pre-biased by tt*C on the host); ref likewise at i >= 16.
        if "gather" not in _skip:
            nc.gpsimd.ap_gather(
                out_ap=gbuf[:, t0:t0 + TB, :], in_ap=lt[:],
                idxs_ap=idx_sb[:, 2 * t0:2 * (t0 + TB)],
                channels=H, num_elems=TB * C, d=1, num_idxs=TB * 32)

    # diag extraction: keep only i == h%16 (hyp) and 16 + h%16 (ref)
    gmask_sb = pp.tile([H, 32], f32)
    nc.sync.dma_start(out=gmask_sb[:], in_=gmask_d[:])
    gm = gmask_sb[:]
    gm_bc = bass.AP(tensor=gm.tensor, offset=gm.offset,
                    ap=[gm.ap[0], [0, NT], gm.ap[1]])
    masked = pp.tile([H, NT, 32], f32)
    nc.vector.tensor_tensor(out=masked[:], in0=gbuf[:], in1=gm_bc,
                            op=Alu.mult)
    hr = pp.tile([H, NT, 2], f32)
    nc.vector.tensor_reduce(
        out=hr[:], in_=masked[:].rearrange("h t (u i) -> h t u i", u=2),
        axis=AxX, op=Alu.add)

    logz = pp.tile([H, NT], f32)
    nc.scalar.activation(out=logz[:], in_=sumexp[:], func=Act.Ln)

    mask_sb = pp.tile([H, 2], f32)
    nc.sync.dma_start(out=mask_sb[:], in_=mask_d[:])
    # mm columns: [sum_h logZ, sum_{h<100} logZ];  gh = sum_h g_hyp;
    # gr = sum_{s<100} g_ref
    mm = psp.tile([NT, 2], f32, space="PSUM")
    nc.tensor.matmul(out=mm[:], lhsT=logz[:], rhs=mask_sb[:],
                     start=True, stop=True)
    gh = psp.tile([NT, 1], f32, space="PSUM")
    nc.tensor.matmul(out=gh[:], lhsT=hr[:, :, 0], rhs=mask_sb[:, 0:1],
                     start=True, stop=True)
    gr = psp.tile([NT, 1], f32, space="PSUM")
    nc.tensor.matmul(out=gr[:], lhsT=hr[:, :, 1], rhs=mask_sb[:, 1:2],
                     start=True, stop=True)

    # lp = Shyp - SlogZ_all ; ce = SlogZ_100 - Sref
    mm_sb = pp.tile([NT, 2], f32)
    nc.vector.tensor_copy(out=mm_sb[:], in_=mm[:])
    nc.vector.tensor_tensor(out=pack[:, 1:2], in0=gh[:], in1=mm_sb[:, 0:1],
                            op=Alu.subtract)
    nc.vector.tensor_tensor(out=pack[:, 2:3], in0=mm_sb[:, 1:2], in1=gr[:],
                            op=Alu.subtract)
    nc.vector.memset(pack[:, 3:4], 0.0)

    # ---------------- per-batch combine ([BL, P] layout) ------------
    fin = pp.tile([BL, P * 4], f32)
    nc.sync.dma_start(out=fin[:], in_=pack[:])
    fv = fin[:].rearrange("b (p k) -> b p k", k=4)
    er_ap, lp_ap, ce_ap = fv[:, :, 0], fv[:, :, 1], fv[:, :, 2]

    mer = pp.tile([BL, 1], f32)
    nc.vector.reduce_sum(out=mer[:], in_=er_ap, axis=AxX)
    nc.vector.tensor_scalar(out=mer[:], in0=mer[:], scalar1=1.0 / P,
                            scalar2=None, op0=Alu.mult)
    erc = pp.tile([BL, P], f32)
    nc.vector.tensor_scalar(out=erc[:], in0=er_ap, scalar1=mer[:],
                            scalar2=None, op0=Alu.subtract)

    negmx = pp.tile([BL, 1], f32)
    nc.vector.tensor_reduce(out=negmx[:], in_=lp_ap, axis=AxX,
                            op=Alu.max, negate=True)
    ew = pp.tile([BL, P], f32)
    se = pp.tile([BL, 1], f32)
    nc.scalar.activation(out=ew[:], in_=lp_ap, func=Act.Exp,
                         bias=negmx[:], scale=1.0, accum_out=se[:])
    inv = pp.tile([BL, 1], f32)
    nc.vector.reciprocal(out=inv[:], in_=se[:])

    t1 = pp.tile([BL, P], f32)
    nc.vector.tensor_tensor(out=t1[:], in0=erc[:], in1=ew[:], op=Alu.mult)
    nc.vector.tensor_scalar(out=t1[:], in0=t1[:], scalar1=inv[:],
                            scalar2=None, op0=Alu.mult)
    contrib = pp.tile([BL, P], f32)
    nc.vector.scalar_tensor_tensor(out=contrib[:], in0=ce_ap,
                                   scalar=0.01, in1=t1[:],
                                   op0=Alu.mult, op1=Alu.add)
    nc.sync.dma_start(out=out_d[:], in_=contrib[:])


def _host_prep(logits, ref, hyp):
    """Build per-core input maps."""
    logits = np.ascontiguousarray(np.asarray(logits, dtype=np.float32))
    ref = np.asarray(ref)
    hyp = np.asarray(hyp)

    mask = np.stack([np.ones(H, np.float32),
                     (np.arange(H) < R).astype(np.float32)], axis=1)
    gmask = np.zeros((H, 32), np.float32)
    hmod = np.arange(H) % 16
    gmask[np.arange(H), hmod] = 1.0
    gmask[np.arange(H), 16 + hmod] = 1.0

    in_maps = []
    for k in range(NCORES):
        sl = slice(k * BL, (k + 1) * BL)
        rf = ref[sl].reshape(NT, R)
        hp = hyp[sl].reshape(NT, H)
        idx16 = np.zeros((H, 2 * NT), np.int16)
        idx16[:, 0::2] = hp.T            # idx16[h, 2t]   = hyp[t, h]
        idx16[:R, 1::2] = rf.T           # idx16[s, 2t+1] = ref[t, s]
        # ap_gather batches 4 tiles: bias each tile's indices into its
        # subtile of the [H, 4*C] input window
        bias = (np.arange(NT) % 4) * C
        idx16[:, 0::2] += bias[None, :].astype(np.int16)
        idx16[:, 1::2] += bias[None, :].astype(np.int16)
        in_maps.append({
            "logits": np.ascontiguousarray(logits[sl].reshape(NT, H, C)),
            "ref_f32": rf.astype(np.float32),
            "hyp_f32": hp.astype(np.float32),
            "idx16": idx16,
            "mask": mask,
            "gmask": gmask,
        })
    return in_maps


def kernel(logits, ref, hyp, _collect=None):
    from concourse import bass_utils

    if "nc" not in _CACHE:
        _CACHE["nc"] = _build_program()
    nc = _CACHE["nc"]

    in_maps = _host_prep(logits, ref, hyp)
    kw = dict(_collect) if _collect else {}
    kw.pop("res", None)
    res = bass_utils.run_bass_kernel_spmd(
        nc, in_maps, core_ids=list(range(NCORES)), **kw)
    if _collect is not None:
        _collect["res"] = res

    total = np.float64(0.0)
    for r in res.results:
        total += np.float64(r["contrib"].astype(np.float64).sum())
    return np.asarray(total / (B * P), dtype=np.float32)


# revision 12
# speedup vs baseline: 1.0558x; 1.0558x over previous
"""Trainium2 Bass kernel for nn_MinimumErrorRateLoss.

Computes, for logits (B,P,H,C), ref (B,P,R), hyp (B,P,H):
    loss = mean_{b,p}[ (er - mean_p er) * softmax_p(log_probs) + 0.01 * ce ]
where
    er        = levenshtein(ref, hyp) / R
    log_probs = sum_h (logits[h, hyp[h]] - logsumexp_c logits[h, :])
    ce        = sum_{s<100} (logsumexp_c logits[s, :] - logits[s, ref[s]])

Sharding: data-parallel over the batch dim across 8 NeuronCores (4 batches
each).  Per core the kernel:
  * streams its 64 (b,p) tiles of [128,1024] logits through SBUF in
    4-tile/2MB DMAs; ScalarE computes exp (no max-subtraction needed for
    ~N(0,1) logits) with a fused free-dim accumulate into a PSUM scratch
    (-> logsumexp after one Ln),
  * extracts the hyp/ref-indexed logits elements with one GPSIMD ap_gather
    per 4-tile batch (per 16-partition group: 16 hyp + 16 ref indices per
    tile, host-pre-biased into the 4-tile window), followed by one
    full-width masked multiply + reduce on VectorE and ones/first-100
    vector matmuls on TensorE (per-element indirect DMA is not viable:
    HW consumes one offset per partition per instruction),
  * runs the edit-distance DP on VectorE in fp16 (all values are small
    integers, exact) with two chained instructions per hyp step (a fused
    scalar_tensor_tensor add and a tensor_tensor_scan max-recurrence),
    using the transformation Y[i,j] = j + i - D[i,j] which turns the row
    update into
        Y_i[j] = max(Y_{i-1}[j-1] + 1 + eq[i,j], Y_i[j-1], Y_{i-1}[j])
    with the boundary column Y_i[0] = 0 for all i, so the scan's initial
    value is the compile-time constant 0.

Measured (8 axon vNCs): relative error 5.1e-07 vs the jax reference;
steady-state ~42-50 us/iteration (reps-slope), vs a ~46 us HBM floor at
the ~735 GB/s effective per-core bandwidth observed on these vNCs.
"""

import numpy as np

B, P, H, R, C = 32, 16, 128, 100, 1024
NCORES = 8
BL = B // NCORES  # local batches per core
NT = BL * P       # tiles (sequences) per core

_CACHE = {}


def _build_program(reps=1, _skip=()):
    import concourse.bass as bass
    import concourse.bacc as bacc
    import concourse.tile as tile
    import concourse.mybir as mybir

    f32 = mybir.dt.float32
    Alu = mybir.AluOpType
    Act = mybir.ActivationFunctionType

    nc = bacc.Bacc("TRN2", target_bir_lowering=False, debug=False)

    logits_d = nc.dram_tensor("logits", [NT, H, C], f32, kind="ExternalInput")
    ref_d = nc.dram_tensor("ref_f32", [NT, R], f32, kind="ExternalInput")
    hyp_d = nc.dram_tensor("hyp_f32", [NT, H], f32, kind="ExternalInput")
    idx_d = nc.dram_tensor("idx16", [H, 2 * NT], mybir.dt.int16,
                           kind="ExternalInput")
    mask_d = nc.dram_tensor("mask", [H, 2], f32, kind="ExternalInput")
    gmask_d = nc.dram_tensor("gmask", [H, 32], f32, kind="ExternalInput")
    out_d = nc.dram_tensor("contrib", [BL, P], f32, kind="ExternalOutput")

    with tile.TileContext(nc) as tc:
        with (
            tc.tile_pool(name="persist", bufs=1) as pp,
            tc.tile_pool(name="lt", bufs=4) as ltp,
            tc.tile_pool(name="scratch", bufs=2, space="PSUM") as scp,
            tc.tile_pool(name="psum", bufs=1, space="PSUM") as psp,
        ):
            for _rep in range(reps):
                _emit_body(nc, bass, mybir, f32, Alu, Act,
                           logits_d, ref_d, hyp_d, idx_d, mask_d, gmask_d,
                           out_d, pp, ltp, scp, psp, _skip)

    nc.compile()
    return nc


def _emit_body(nc, bass, mybir, f32, Alu, Act,
               logits_d, ref_d, hyp_d, idx_d, mask_d, gmask_d, out_d,
               pp, ltp, scp, psp, _skip=()):
    AxX = mybir.AxisListType.X

    # ---------------- DP inputs and serial chain (VectorE) ----------
    ref_sb = pp.tile([NT, R], f32)
    hyp_sb = pp.tile([NT, H], f32)
    nc.sync.dma_start(out=ref_sb[:], in_=ref_d[:])
    nc.sync.dma_start(out=hyp_sb[:], in_=hyp_d[:])

    f16 = mybir.dt.float16
    eqm = pp.tile([NT, H, R], f16)
    ra, ha = ref_sb[:], hyp_sb[:]
    # eqm[t, s, j] = (ref[t, j] == hyp[t, s])
    ref_bc = bass.AP(tensor=ra.tensor, offset=ra.offset,
                     ap=[ra.ap[0], [0, H], ra.ap[1]])
    hyp_bc = bass.AP(tensor=ha.tensor, offset=ha.offset,
                     ap=[ha.ap[0], ha.ap[1], [0, R]])
    if "eq" not in _skip:
        nc.vector.tensor_tensor(out=eqm[:], in0=ref_bc, in1=hyp_bc,
                                op=Alu.is_equal)
    else:
        nc.vector.memset(eqm[:], 0.0)

    ya = pp.tile([NT, R + 1], f16)
    yb = pp.tile([NT, R + 1], f16)
    ab = pp.tile([NT, R], f16)
    nc.vector.memset(ya[:], 0.0)
    nc.vector.memset(yb[:, 0:1], 0.0)

    bufs = [ya, yb]
    for s in range(H if "dp" not in _skip else 0):
        yp = bufs[s % 2]
        yn = bufs[(s + 1) % 2]
        # A[j] = Yprev[j-1] + 1 + eq[s, j],   j = 1..R
        nc.vector.scalar_tensor_tensor(
            out=ab[:], in0=yp[:, 0:R], scalar=1.0, in1=eqm[:, s, :],
            op0=Alu.add, op1=Alu.add)
        # Ynew[j] = max(A[j], Ynew[j-1], Yprev[j]),  Ynew[0] = 0
        nc.vector.tensor_tensor_scan(
            out=yn[:, 1:R + 1], data0=ab[:], data1=yp[:, 1:R + 1],
            initial=0.0, op0=Alu.max, op1=Alu.max)

    yfin = bufs[H % 2]
    pack = pp.tile([NT, 4], f32)
    # er = dist/R = (R + H - Y)/R
    nc.vector.tensor_scalar(
        out=pack[:, 0:1], in0=yfin[:, R:R + 1],
        scalar1=-1.0 / R, scalar2=float(R + H) / R,
        op0=Alu.mult, op1=Alu.add)

    # ------------- logsumexp stream + per-tile gathers ---------------
    idx_sb = pp.tile([H, 2 * NT], mybir.dt.int16)
    nc.sync.dma_start(out=idx_sb[:], in_=idx_d[:])
    gbuf = pp.tile([H, NT, 32], f32)
    sumexp = pp.tile([H, NT], f32)
    if "gather" in _skip:
        nc.vector.memset(gbuf[:], 0.0)
    TB = 8  # tiles per DMA / ap_gather batch
    lgap = logits_d.ap()
    for t0 in range(0, NT, TB):
        lt = ltp.tile([H, TB, C], f32)
        # DRAM [t, h, c] -> SBUF [h, t, c]
        src_ap = bass.AP(tensor=lgap.tensor, offset=t0 * H * C,
                         ap=[[C, H], [H * C, TB], [1, C]])
        nc.sync.dma_start(out=lt[:], in_=src_ap)
        for tt in range(TB):
            t = t0 + tt
            if "act" not in _skip:
                sc = scp.tile([H, C], f32, space="PSUM")
                nc.scalar.activation(out=sc[:], in_=lt[:, tt, :],
                                     func=Act.Exp,
                                     accum_out=sumexp[:, t:t + 1])
            elif t == 0:
                nc.vector.memset(sumexp[:], 1.0)
        # out[h, tt, i<16] = lt[h, tt, hyp[t0+tt, 16*(h//16)+i]]  (idxs
        # pre-biased by tt*C on the host); ref likewise at i >= 16.
        if "gather" not in _skip:
            nc.gpsimd.ap_gather(
                out_ap=gbuf[:, t0:t0 + TB, :], in_ap=lt[:],
                idxs_ap=idx_sb[:, 2 * t0:2 * (t0 + TB)],
                channels=H, num_elems=TB * C, d=1, num_idxs=TB * 32)

    # diag extraction: keep only i == h%16 (hyp) and 16 + h%16 (ref)
    gmask_sb = pp.tile([H, 32], f32)
    nc.sync.dma_start(out=gmask_sb[:], in_=gmask_d[:])
    gm = gmask_sb[:]
    gm_bc = bass.AP(tensor=gm.tensor, offset=gm.offset,
                    ap=[gm.ap[0], [0, NT], gm.ap[1]])
    masked = pp.tile([H, NT, 32], f32)
    nc.vector.tensor_tensor(out=masked[:], in0=gbuf[:], in1=gm_bc,
                            op=Alu.mult)
    hr = pp.tile([H, NT, 2], f32)
    nc.vector.tensor_reduce(
        out=hr[:], in_=masked[:].rearrange("h t (u i) -> h t u i", u=2),
        axis=AxX, op=Alu.add)

    logz = pp.tile([H, NT], f32)
    nc.scalar.activation(out=logz[:], in_=sumexp[:], func=Act.Ln)

    mask_sb = pp.tile([H, 2], f32)
    nc.sync.dma_start(out=mask_sb[:], in_=mask_d[:])
    # mm columns: [sum_h logZ, sum_{h<100} logZ];  gh = sum_h g_hyp;
    # gr = sum_{s<100} g_ref
    mm = psp.tile([NT, 2], f32, space="PSUM")
    nc.tensor.matmul(out=mm[:], lhsT=logz[:], rhs=mask_sb[:],
                     start=True, stop=True)
    gh = psp.tile([NT, 1], f32, space="PSUM")
    nc.tensor.matmul(out=gh[:], lhsT=hr[:, :, 0], rhs=mask_sb[:, 0:1],
                     start=True, stop=True)
    gr = psp.tile([NT, 1], f32, space="PSUM")
    nc.tensor.matmul(out=gr[:], lhsT=hr[:, :, 1], rhs=mask_sb[:, 1:2],
                     start=True, stop=True)

    # lp = Shyp - SlogZ_all ; ce = SlogZ_100 - Sref
    mm_sb = pp.tile([NT, 2], f32)
    nc.vector.tensor_copy(out=mm_sb[:], in_=mm[:])
    nc.vector.tensor_tensor(out=pack[:, 1:2], in0=gh[:], in1=mm_sb[:, 0:1],
                            op=Alu.subtract)
    nc.vector.tensor_tensor(out=pack[:, 2:3], in0=mm_sb[:, 1:2], in1=gr[:],
                            op=Alu.subtract)
    nc.vector.memset(pack[:, 3:4], 0.0)

    # ---------------- per-batch combine ([BL, P] layout) ------------
    fin = pp.tile([BL, P * 4], f32)
    nc.sync.dma_start(out=fin[:], in_=pack[:])
    fv = fin[:].rearrange("b (p k) -> b p k", k=4)
    er_ap, lp_ap, ce_ap = fv[:, :, 0], fv[:, :, 1], fv[:, :, 2]

    mer = pp.tile([BL, 1], f32)
    nc.vector.reduce_sum(out=mer[:], in_=er_ap, axis=AxX)
    nc.vector.tensor_scalar(out=mer[:], in0=mer[:], scalar1=1.0 / P,
                            scalar2=None, op0=Alu.mult)
    erc = pp.tile([BL, P], f32)
    nc.vector.tensor_scalar(out=erc[:], in0=er_ap, scalar1=mer[:],
                            scalar2=None, op0=Alu.subtract)

    negmx = pp.tile([BL, 1], f32)
    nc.vector.tensor_reduce(out=negmx[:], in_=lp_ap, axis=AxX,
                            op=Alu.max, negate=True)
    ew = pp.tile([BL, P], f32)
    se = pp.tile([BL, 1], f32)
    nc.scalar.activation(out=ew[:], in_=lp_ap, func=Act.Exp,
                         bias=negmx[:], scale=1.0, accum_out=se[:])
    inv = pp.tile([BL, 1], f32)
    nc.vector.reciprocal(out=inv[:], in_=se[:])

    t1 = pp.tile([BL, P], f32)
    nc.vector.tensor_tensor(out=t1[:], in0=erc[:], in1=ew[:], op=Alu.mult)
    nc.vector.tensor_scalar(out=t1[:], in0=t1[:], scalar1=inv[:],
                            scalar2=None, op0=Alu.mult)
    contrib = pp.tile([BL, P], f32)
    nc.vector.scalar_tensor_tensor(out=contrib[:], in0=ce_ap,
                                   scalar=0.01, in1=t1[:],
                                   op0=Alu.mult, op1=Alu.add)
    nc.sync.dma_start(out=out_d[:], in_=contrib[:])


def _host_prep(logits, ref, hyp):
    """Build per-core input maps."""
    logits = np.ascontiguousarray(np.asarray(logits, dtype=np.float32))
    ref = np.asarray(ref)
    hyp = np.asarray(hyp)

    mask = np.stack([np.ones(H, np.float32),
                     (np.arange(H) < R).astype(np.float32)], axis=1)
    gmask = np.zeros((H, 32), np.float32)
    hmod = np.arange(H) % 16
    gmask[np.arange(H), hmod] = 1.0
    gmask[np.arange(H), 16 + hmod] = 1.0

    in_maps = []
    for k in range(NCORES):
        sl = slice(k * BL, (k + 1) * BL)
        rf = ref[sl].reshape(NT, R)
        hp = hyp[sl].reshape(NT, H)
        idx16 = np.zeros((H, 2 * NT), np.int16)
        idx16[:, 0::2] = hp.T            # idx16[h, 2t]   = hyp[t, h]
        idx16[:R, 1::2] = rf.T           # idx16[s, 2t+1] = ref[t, s]
        # ap_gather batches 4 tiles: bias each tile's indices into its
        # subtile of the [H, 4*C] input window
        bias = (np.arange(NT) % 8) * C
        idx16[:, 0::2] += bias[None, :].astype(np.int16)
        idx16[:, 1::2] += bias[None, :].astype(np.int16)
        in_maps.append({
            "logits": np.ascontiguousarray(logits[sl].reshape(NT, H, C)),
            "ref_f32": rf.astype(np.float32),
            "hyp_f32": hp.astype(np.float32),
            "idx16": idx16,
            "mask": mask,
            "gmask": gmask,
        })
    return in_maps


def kernel(logits, ref, hyp, _collect=None):
    from concourse import bass_utils

    if "nc" not in _CACHE:
        _CACHE["nc"] = _build_program()
    nc = _CACHE["nc"]

    in_maps = _host_prep(logits, ref, hyp)
    kw = dict(_collect) if _collect else {}
    kw.pop("res", None)
    res = bass_utils.run_bass_kernel_spmd(
        nc, in_maps, core_ids=list(range(NCORES)), **kw)
    if _collect is not None:
        _collect["res"] = res

    total = np.float64(0.0)
    for r in res.results:
        total += np.float64(r["contrib"].astype(np.float64).sum())
    return np.asarray(total / (B * P), dtype=np.float32)


# revision 13
# speedup vs baseline: 1.2951x; 1.2267x over previous
"""Trainium2 Bass kernel for nn_MinimumErrorRateLoss.

Computes, for logits (B,P,H,C), ref (B,P,R), hyp (B,P,H):
    loss = mean_{b,p}[ (er - mean_p er) * softmax_p(log_probs) + 0.01 * ce ]
where
    er        = levenshtein(ref, hyp) / R
    log_probs = sum_h (logits[h, hyp[h]] - logsumexp_c logits[h, :])
    ce        = sum_{s<100} (logsumexp_c logits[s, :] - logits[s, ref[s]])

Sharding: data-parallel over the batch dim across 8 NeuronCores (4 batches
each).  Per core the kernel:
  * streams its 64 (b,p) tiles of [128,1024] logits through SBUF in
    4-tile/2MB DMAs; ScalarE computes exp (no max-subtraction needed for
    ~N(0,1) logits) with a fused free-dim accumulate into a PSUM scratch
    (-> logsumexp after one Ln),
  * extracts the hyp/ref-indexed logits elements with one GPSIMD ap_gather
    per 4-tile batch (per 16-partition group: 16 hyp + 16 ref indices per
    tile, host-pre-biased into the 4-tile window), followed by one
    full-width masked multiply + reduce on VectorE and ones/first-100
    vector matmuls on TensorE (per-element indirect DMA is not viable:
    HW consumes one offset per partition per instruction),
  * runs the edit-distance DP on VectorE in fp16 (all values are small
    integers, exact) with two chained instructions per hyp step (a fused
    scalar_tensor_tensor add and a tensor_tensor_scan max-recurrence),
    using the transformation Y[i,j] = j + i - D[i,j] which turns the row
    update into
        Y_i[j] = max(Y_{i-1}[j-1] + 1 + eq[i,j], Y_i[j-1], Y_{i-1}[j])
    with the boundary column Y_i[0] = 0 for all i, so the scan's initial
    value is the compile-time constant 0.

Measured (8 axon vNCs): relative error 5.1e-07 vs the jax reference;
steady-state ~42-50 us/iteration (reps-slope), vs a ~46 us HBM floor at
the ~735 GB/s effective per-core bandwidth observed on these vNCs.
"""

import numpy as np

B, P, H, R, C = 32, 16, 128, 100, 1024
NCORES = 8
BL = B // NCORES  # local batches per core
NT = BL * P       # tiles (sequences) per core

_CACHE = {}


def _build_program(reps=1, _skip=()):
    import concourse.bass as bass
    import concourse.bacc as bacc
    import concourse.tile as tile
    import concourse.mybir as mybir

    f32 = mybir.dt.float32
    Alu = mybir.AluOpType
    Act = mybir.ActivationFunctionType

    nc = bacc.Bacc("TRN2", target_bir_lowering=False, debug=False)

    logits_d = nc.dram_tensor("logits", [NT, H, C], f32, kind="ExternalInput")
    ref_d = nc.dram_tensor("ref_f32", [NT, R], f32, kind="ExternalInput")
    hyp_d = nc.dram_tensor("hyp_f32", [NT, H], f32, kind="ExternalInput")
    idx_d = nc.dram_tensor("idx16", [H, 2 * NT], mybir.dt.int16,
                           kind="ExternalInput")
    mask_d = nc.dram_tensor("mask", [H, 2], f32, kind="ExternalInput")
    gmask_d = nc.dram_tensor("gmask", [H, 32], f32, kind="ExternalInput")
    out_d = nc.dram_tensor("contrib", [BL, P], f32, kind="ExternalOutput")

    with tile.TileContext(nc) as tc:
        with (
            tc.tile_pool(name="persist", bufs=1) as pp,
            tc.tile_pool(name="lt", bufs=4) as ltp,
            tc.tile_pool(name="scratch", bufs=2, space="PSUM") as scp,
            tc.tile_pool(name="psum", bufs=1, space="PSUM") as psp,
        ):
            for _rep in range(reps):
                _emit_body(nc, bass, mybir, f32, Alu, Act,
                           logits_d, ref_d, hyp_d, idx_d, mask_d, gmask_d,
                           out_d, pp, ltp, scp, psp, _skip)

    nc.compile()
    return nc


def _emit_body(nc, bass, mybir, f32, Alu, Act,
               logits_d, ref_d, hyp_d, idx_d, mask_d, gmask_d, out_d,
               pp, ltp, scp, psp, _skip=()):
    AxX = mybir.AxisListType.X

    # ---------------- DP inputs and serial chain (VectorE) ----------
    ref_sb = pp.tile([NT, R], f32)
    hyp_sb = pp.tile([NT, H], f32)
    nc.sync.dma_start(out=ref_sb[:], in_=ref_d[:])
    nc.sync.dma_start(out=hyp_sb[:], in_=hyp_d[:])

    f16 = mybir.dt.float16
    eqm = pp.tile([NT, H, R], f16)
    ra, ha = ref_sb[:], hyp_sb[:]
    # eqm[t, s, j] = (ref[t, j] == hyp[t, s])
    ref_bc = bass.AP(tensor=ra.tensor, offset=ra.offset,
                     ap=[ra.ap[0], [0, H], ra.ap[1]])
    hyp_bc = bass.AP(tensor=ha.tensor, offset=ha.offset,
                     ap=[ha.ap[0], ha.ap[1], [0, R]])
    if "eq" not in _skip:
        nc.vector.tensor_tensor(out=eqm[:], in0=ref_bc, in1=hyp_bc,
                                op=Alu.is_equal)
    else:
        nc.vector.memset(eqm[:], 0.0)

    ya = pp.tile([NT, R + 1], f16)
    yb = pp.tile([NT, R + 1], f16)
    ab = pp.tile([NT, R], f16)
    nc.vector.memset(ya[:], 0.0)
    nc.vector.memset(yb[:, 0:1], 0.0)

    bufs = [ya, yb]
    for s in range(H if "dp" not in _skip else 0):
        yp = bufs[s % 2]
        yn = bufs[(s + 1) % 2]
        # A[j] = Yprev[j-1] + 1 + eq[s, j],   j = 1..R
        nc.vector.scalar_tensor_tensor(
            out=ab[:], in0=yp[:, 0:R], scalar=1.0, in1=eqm[:, s, :],
            op0=Alu.add, op1=Alu.add)
        # Ynew[j] = max(A[j], Ynew[j-1], Yprev[j]),  Ynew[0] = 0
        nc.vector.tensor_tensor_scan(
            out=yn[:, 1:R + 1], data0=ab[:], data1=yp[:, 1:R + 1],
            initial=0.0, op0=Alu.max, op1=Alu.max)

    yfin = bufs[H % 2]
    pack = pp.tile([NT, 4], f32)
    # er = dist/R = (R + H - Y)/R
    nc.vector.tensor_scalar(
        out=pack[:, 0:1], in0=yfin[:, R:R + 1],
        scalar1=-1.0 / R, scalar2=float(R + H) / R,
        op0=Alu.mult, op1=Alu.add)

    # ------------- logsumexp stream + per-tile gathers ---------------
    idx_sb = pp.tile([H, 2 * NT], mybir.dt.int16)
    nc.sync.dma_start(out=idx_sb[:], in_=idx_d[:])
    gbuf = pp.tile([H, NT, 32], f32)
    sumexp = pp.tile([H, NT], f32)
    if "gather" in _skip:
        nc.vector.memset(gbuf[:], 0.0)
    TB = 4  # tiles per DMA / ap_gather batch
    lgap = logits_d.ap()
    for t0 in range(0, NT, TB):
        lt = ltp.tile([H, TB, C], f32)
        # DRAM [t, h, c] -> SBUF [h, t, c]
        src_ap = bass.AP(tensor=lgap.tensor, offset=t0 * H * C,
                         ap=[[C, H], [H * C, TB], [1, C]])
        nc.sync.dma_start(out=lt[:], in_=src_ap)
        for tt in range(TB):
            t = t0 + tt
            if "act" not in _skip:
                sc = scp.tile([H, C], f32, space="PSUM")
                nc.scalar.activation(out=sc[:], in_=lt[:, tt, :],
                                     func=Act.Exp,
                                     accum_out=sumexp[:, t:t + 1])
            elif t == 0:
                nc.vector.memset(sumexp[:], 1.0)
        # out[h, tt, i<16] = lt[h, tt, hyp[t0+tt, 16*(h//16)+i]]  (idxs
        # pre-biased by tt*C on the host); ref likewise at i >= 16.
        if "gather" not in _skip:
            nc.gpsimd.ap_gather(
                out_ap=gbuf[:, t0:t0 + TB, :], in_ap=lt[:],
                idxs_ap=idx_sb[:, 2 * t0:2 * (t0 + TB)],
                channels=H, num_elems=TB * C, d=1, num_idxs=TB * 32)

    # diag extraction: keep only i == h%16 (hyp) and 16 + h%16 (ref)
    gmask_sb = pp.tile([H, 32], f32)
    nc.sync.dma_start(out=gmask_sb[:], in_=gmask_d[:])
    gm = gmask_sb[:]
    gm_bc = bass.AP(tensor=gm.tensor, offset=gm.offset,
                    ap=[gm.ap[0], [0, NT], gm.ap[1]])
    masked = pp.tile([H, NT, 32], f32)
    nc.vector.tensor_tensor(out=masked[:], in0=gbuf[:], in1=gm_bc,
                            op=Alu.mult)
    hr = pp.tile([H, NT, 2], f32)
    nc.vector.tensor_reduce(
        out=hr[:], in_=masked[:].rearrange("h t (u i) -> h t u i", u=2),
        axis=AxX, op=Alu.add)

    logz = pp.tile([H, NT], f32)
    nc.scalar.activation(out=logz[:], in_=sumexp[:], func=Act.Ln)

    mask_sb = pp.tile([H, 2], f32)
    nc.sync.dma_start(out=mask_sb[:], in_=mask_d[:])
    # mm columns: [sum_h logZ, sum_{h<100} logZ];  gh = sum_h g_hyp;
    # gr = sum_{s<100} g_ref
    mm = psp.tile([NT, 2], f32, space="PSUM")
    nc.tensor.matmul(out=mm[:], lhsT=logz[:], rhs=mask_sb[:],
                     start=True, stop=True)
    gh = psp.tile([NT, 1], f32, space="PSUM")
    nc.tensor.matmul(out=gh[:], lhsT=hr[:, :, 0], rhs=mask_sb[:, 0:1],
                     start=True, stop=True)
    gr = psp.tile([NT, 1], f32, space="PSUM")
    nc.tensor.matmul(out=gr[:], lhsT=hr[:, :, 1], rhs=mask_sb[:, 1:2],
                     start=True, stop=True)

    # lp = Shyp - SlogZ_all ; ce = SlogZ_100 - Sref
    mm_sb = pp.tile([NT, 2], f32)
    nc.vector.tensor_copy(out=mm_sb[:], in_=mm[:])
    nc.vector.tensor_tensor(out=pack[:, 1:2], in0=gh[:], in1=mm_sb[:, 0:1],
                            op=Alu.subtract)
    nc.vector.tensor_tensor(out=pack[:, 2:3], in0=mm_sb[:, 1:2], in1=gr[:],
                            op=Alu.subtract)
    nc.vector.memset(pack[:, 3:4], 0.0)

    # ---------------- per-batch combine ([BL, P] layout) ------------
    fin = pp.tile([BL, P * 4], f32)
    nc.sync.dma_start(out=fin[:], in_=pack[:])
    fv = fin[:].rearrange("b (p k) -> b p k", k=4)
    er_ap, lp_ap, ce_ap = fv[:, :, 0], fv[:, :, 1], fv[:, :, 2]

    mer = pp.tile([BL, 1], f32)
    nc.vector.reduce_sum(out=mer[:], in_=er_ap, axis=AxX)
    nc.vector.tensor_scalar(out=mer[:], in0=mer[:], scalar1=1.0 / P,
                            scalar2=None, op0=Alu.mult)
    erc = pp.tile([BL, P], f32)
    nc.vector.tensor_scalar(out=erc[:], in0=er_ap, scalar1=mer[:],
                            scalar2=None, op0=Alu.subtract)

    negmx = pp.tile([BL, 1], f32)
    nc.vector.tensor_reduce(out=negmx[:], in_=lp_ap, axis=AxX,
                            op=Alu.max, negate=True)
    ew = pp.tile([BL, P], f32)
    se = pp.tile([BL, 1], f32)
    nc.scalar.activation(out=ew[:], in_=lp_ap, func=Act.Exp,
                         bias=negmx[:], scale=1.0, accum_out=se[:])
    inv = pp.tile([BL, 1], f32)
    nc.vector.reciprocal(out=inv[:], in_=se[:])

    t1 = pp.tile([BL, P], f32)
    nc.vector.tensor_tensor(out=t1[:], in0=erc[:], in1=ew[:], op=Alu.mult)
    nc.vector.tensor_scalar(out=t1[:], in0=t1[:], scalar1=inv[:],
                            scalar2=None, op0=Alu.mult)
    contrib = pp.tile([BL, P], f32)
    nc.vector.scalar_tensor_tensor(out=contrib[:], in0=ce_ap,
                                   scalar=0.01, in1=t1[:],
                                   op0=Alu.mult, op1=Alu.add)
    nc.sync.dma_start(out=out_d[:], in_=contrib[:])


def _host_prep(logits, ref, hyp):
    """Build per-core input maps."""
    logits = np.ascontiguousarray(np.asarray(logits, dtype=np.float32))
    ref = np.asarray(ref)
    hyp = np.asarray(hyp)

    mask = np.stack([np.ones(H, np.float32),
                     (np.arange(H) < R).astype(np.float32)], axis=1)
    gmask = np.zeros((H, 32), np.float32)
    hmod = np.arange(H) % 16
    gmask[np.arange(H), hmod] = 1.0
    gmask[np.arange(H), 16 + hmod] = 1.0

    in_maps = []
    for k in range(NCORES):
        sl = slice(k * BL, (k + 1) * BL)
        rf = ref[sl].reshape(NT, R)
        hp = hyp[sl].reshape(NT, H)
        idx16 = np.zeros((H, 2 * NT), np.int16)
        idx16[:, 0::2] = hp.T            # idx16[h, 2t]   = hyp[t, h]
        idx16[:R, 1::2] = rf.T           # idx16[s, 2t+1] = ref[t, s]
        # ap_gather batches 4 tiles: bias each tile's indices into its
        # subtile of the [H, 4*C] input window
        bias = (np.arange(NT) % 4) * C
        idx16[:, 0::2] += bias[None, :].astype(np.int16)
        idx16[:, 1::2] += bias[None, :].astype(np.int16)
        in_maps.append({
            "logits": np.ascontiguousarray(logits[sl].reshape(NT, H, C)),
            "ref_f32": rf.astype(np.float32),
            "hyp_f32": hp.astype(np.float32),
            "idx16": idx16,
            "mask": mask,
            "gmask": gmask,
        })
    return in_maps


def kernel(logits, ref, hyp, _collect=None):
    from concourse import bass_utils

    if "nc" not in _CACHE:
        _CACHE["nc"] = _build_program()
    nc = _CACHE["nc"]

    in_maps = _host_prep(logits, ref, hyp)
    kw = dict(_collect) if _collect else {}
    kw.pop("res", None)
    res = bass_utils.run_bass_kernel_spmd(
        nc, in_maps, core_ids=list(range(NCORES)), **kw)
    if _collect is not None:
        _collect["res"] = res

    total = np.float64(0.0)
    for r in res.results:
        total += np.float64(r["contrib"].astype(np.float64).sum())
    return np.asarray(total / (B * P), dtype=np.float32)
